# revision 42
# baseline (speedup 1.0000x reference)
"""Causal multi-head attention (B=4, N=4, L=1024, H=8, E=64) on 8 trn2 cores.

Sharding: the 16 (b, n) pairs are split 2-per-core (batch/head-group data
parallelism) -- each core runs the identical Bass program on its own slice,
no cross-core communication.

Design notes:
  - Inputs converted to bf16 on the host: halves input DMA and keeps all
    matmuls at full PE rate.
  - Q^T/K^T slabs are loaded straight from the natural [l, (h,e)] DRAM
    layout with XBAR DMA-transposes (dma_start_transpose) -- no PE input
    transposes, no PSUM staging, no per-row DMA descriptors.
  - QK matmuls of the two heads of a head-pair sit on disjoint PE row
    groups (contract dim 64 at base partitions 0/64) so consecutive
    hh0/hh1 matmuls execute CONCURRENTLY in the array (row tiling).
  - DMA dispatch time (~8ns/descriptor on the issuing sequencer) is a
    first-order cost: V is loaded in one whole-pair DMA and each
    super-unit writes one batched output DMA (512B descriptors).
  - exp is split across engines (gpsimd cannot touch PSUM): head hh0,
    qg0, and sk-trimmed tiles get exact ACT exp; head hh1's wide qg1
    tiles use a one-op Schraudolph exp2 on DVE (tensor_scalar to int16,
    bitcast bf16; ~3.4% max exp err, ~0.4% end-to-end here).
  - Epilogue: av [65, 512] is copied to SBUF immediately by DVE (frees
    the PSUM bank so the next super-unit's AV matmuls never wait on a
    DMA chain), PE-transposed back to [q, e] with the softmax
    denominators riding along as the 65th column, normalized with a
    4-element/lane reciprocal + broadcast multiply, and stored.
"""

import sys

if "/opt/trn_rl_repo" not in sys.path:
    sys.path.insert(0, "/opt/trn_rl_repo")

import numpy as np

_CACHE = {}

B, N, L, H, E = 4, 4, 1024, 8, 64
CORES = 8
PAIRS = (B * N) // CORES  # (b,n) pairs per core
ROWS = PAIRS * L  # dram rows per core
HE = H * E
LT = L // 128  # 128-row l-tiles per pair

# Schraudolph exp2 constants (bf16 bit pattern via int16):
# exp(x) ~ bitcast_bf16(int16(x * (2^7/ln2) + (127*2^7 - C))), C tuned for
# minimal relative error (measured ~3.4% max on hw). The 1/8 softmax scale
# is folded into the multiplier.
SCHR_A = (2.0**7) / np.log(2.0) * 0.125
SCHR_B = 127.0 * 2**7 - 6.0


def _build(reps=1):
    key = ("nc", reps)
    if key in _CACHE:
        return _CACHE[key]

    import ml_dtypes
    import concourse.bass as bass
    import concourse.tile as tile
    from concourse import bacc, mybir

    f32 = mybir.dt.float32
    i16 = mybir.dt.int16
    bf16 = mybir.dt.bfloat16
    np_bf16 = ml_dtypes.bfloat16
    AF = mybir.ActivationFunctionType
    ALU = mybir.AluOpType

    nc = bacc.Bacc("TRN2", target_bir_lowering=False, debug=False, num_devices=CORES)
    qd = nc.dram_tensor("queries", [ROWS, HE], bf16, kind="ExternalInput").ap()
    kd = nc.dram_tensor("keys", [ROWS, HE], bf16, kind="ExternalInput").ap()
    vd = nc.dram_tensor("values", [ROWS, HE], bf16, kind="ExternalInput").ap()
    od = nc.dram_tensor("out", [ROWS, HE], f32, kind="ExternalOutput").ap()

    # Triangle mask: mask_np[k, c] = 1.0 iff c >= k.
    cols = np.arange(128)[None, :]
    rows = np.arange(128)[:, None]
    mask_np = (cols >= rows).astype(np_bf16)
    maskd = nc.inline_tensor(mask_np, name="cmasks").ap()
    identd = nc.inline_tensor(np.eye(128, dtype=np_bf16), name="ident").ap()
    onesd = nc.inline_tensor(np.ones((128, 1), dtype=np_bf16), name="ones").ap()

    with tile.TileContext(nc) as tc:
        with (
            tc.tile_pool(name="const", bufs=1) as cpool,
            tc.tile_pool(name="load", bufs=6) as lpool,
            tc.tile_pool(name="qt", bufs=2) as qtpool,
            tc.tile_pool(name="kt", bufs=2) as ktpool,
            tc.tile_pool(name="vp", bufs=2) as vppool,
            tc.tile_pool(name="es", bufs=18) as espool,
            tc.tile_pool(name="t1", bufs=4) as t1pool,
            tc.tile_pool(name="o", bufs=3) as opool,
            tc.tile_pool(name="r", bufs=4) as rpool,
            tc.tile_pool(name="ps_s", bufs=2, space="PSUM") as pss,
            tc.tile_pool(name="ps_av", bufs=2, space="PSUM") as psav,
            tc.tile_pool(name="ps_t", bufs=2, space="PSUM") as pst,
        ):
            ident = cpool.tile([128, 128], bf16)
            nc.sync.dma_start(ident[:, :], identd[:, :])
            ones = cpool.tile([128, 1], bf16)
            nc.sync.dma_start(ones[:, :], onesd[:, :])
            masks = cpool.tile([128, 128], bf16)

            slabs = {}

            def alloc_slab(pair):
                qt = qtpool.tile([128, 4, L], bf16, tag="qt")
                kt = ktpool.tile([128, 4, L], bf16, tag="kt")
                vp = vppool.tile([128, LT, H, E + 1], bf16, tag="vp")
                nc.gpsimd.tensor_copy(
                    vp[:, :, :, E : E + 1],
                    ones.broadcast_to([128, LT, H, 1]),
                )
                slabs[pair] = (qt, kt, vp)

            def emit_slab_qk(pair):
                # XBAR DMA-transpose: loads Q^T/K^T [(hh,e), l] directly from
                # the natural [l, (h,e)] DRAM layout -- no PE transposes, no
                # PSUM staging, no per-row descriptors.
                qt, kt, _ = slabs[pair]
                r0 = pair * L
                for pr in range(4):
                    nc.sync.dma_start_transpose(
                        qt[:, pr, :], qd[r0 : r0 + L, pr * 128 : (pr + 1) * 128]
                    )
                    nc.sync.dma_start_transpose(
                        kt[:, pr, :], kd[r0 : r0 + L, pr * 128 : (pr + 1) * 128]
                    )

            def emit_slab_v(pair):
                _, _, vp = slabs[pair]
                r0 = pair * L
                vload = lpool.tile([128, LT, HE], bf16, tag="ld")
                nc.sync.dma_start(
                    vload[:, :, :],
                    vd[r0 : r0 + L, :].rearrange("(t p) e -> p t e", p=128),
                )
                nc.gpsimd.tensor_copy(
                    vp[:, :, :, 0:E],
                    vload.rearrange("p t (h e) -> p t h e", e=E),
                )

            def emit_qk_exp(pair, hp, qg, jp):
                qt, kt, vp = slabs[pair]
                tp0 = 2 * jp - 4 * qg
                sk = 128 * tp0 if tp0 > 0 else 0
                s0 = pss.tile([128, 1024], f32, tag="s")
                s1 = pss.tile([128, 1024], f32, tag="s")
                ss = [s0, s1]
                # hh0/hh1 interleaved: disjoint PE row groups run
                # concurrently (row tiling)
                for half in range(2):
                    j = 2 * jp + half
                    for hh in range(2):
                        lhsT = kt[
                            64 * hh : 64 * hh + 64, hp, j * 128 : (j + 1) * 128
                        ]
                        rhs = qt[
                            64 * hh : 64 * hh + 64,
                            hp,
                            qg * 512 + sk : (qg + 1) * 512,
                        ]
                        nc.tensor.matmul(
                            ss[hh][:, half * 512 + sk : (half + 1) * 512],
                            lhsT,
                            rhs,
                            start=True,
                            stop=True,
                        )
                pes = []
                for hh in range(2):
                    es = espool.tile([128, 1024], bf16, tag="es")
                    sv = ss[hh].rearrange("p (u c) -> p u c", u=2)[:, :, sk:512]
                    ev = es.rearrange("p (u c) -> p u c", u=2)[:, :, sk:512]
                    # gpsimd cannot read PSUM, so exp runs on ACT + DVE:
                    # head hh0, qg0, and the narrow sk-trimmed tiles use
                    # exact ACT exp; head hh1's wide qg1 tiles use the
                    # one-op Schraudolph exp2 on DVE (load balance).
                    if hh == 0 or sk > 0 or qg == 0:
                        nc.scalar.activation(ev, sv, AF.Exp, scale=0.125)
                    else:
                        nc.vector.tensor_scalar(
                            ev.bitcast(i16),
                            sv,
                            SCHR_A,
                            SCHR_B,
                            ALU.mult,
                            ALU.add,
                        )
                    pes.append(es)
                t0 = 2 * jp - 4 * qg
                if t0 >= 0:
                    # Diagonal pair: mask the two [128,127] triangles.
                    c0 = 128 * t0

                    def tri(ap, off=c0):
                        return bass.AP(
                            ap.tensor,
                            ap.offset + off,
                            [list(ap.ap[0]), [640, 2], [1, 127]],
                        )

                    mb = bass.AP(
                        masks.tensor,
                        masks.offset,
                        [list(masks.ap[0]), [0, 2], [1, 127]],
                    )
                    nc.gpsimd.tensor_mul(tri(pes[0]), tri(pes[0]), mb)
                    nc.gpsimd.tensor_mul(tri(pes[1]), tri(pes[1]), mb)
                return pes

            def emit_av(pair, hp, qg, avs, pes, jp):
                _, _, vp = slabs[pair]
                jn = 4 * qg + 4
                for half in range(2):
                    j = 2 * jp + half
                    t = j - 4 * qg
                    c0 = 128 * t if t > 0 else 0
                    for hh in range(2):
                        nc.tensor.matmul(
                            avs[hh][:, c0:512],
                            vp[:, j, 2 * hp + hh, :],
                            pes[hh][:, half * 512 + c0 : (half + 1) * 512],
                            start=(j == 0),
                            stop=(j == jn - 1),
                            skip_group_check=True,
                        )

            def unit_epilogue(pair, hp, qg, avs):
                # copy av out of PSUM right away (shortest possible chain on
                # the psum bank), transpose back to [q, e] with denominators
                # in the 65th column, normalize, one batched store.
                t1s = []
                for hh in range(2):
                    t1 = t1pool.tile([E + 1, 512], bf16, tag="t1")
                    nc.vector.tensor_copy(t1[:, :], avs[hh][:, :])
                    t1s.append(t1)
                o = opool.tile([128, 4, 2, E], f32, tag="o")
                for hh in range(2):
                    # inner dim padded to E+2 so each bf16 PSUM slice starts
                    # 4-byte aligned (walrus checkMatmultOutputs)
                    ot = pst.tile([128, 4, E + 2], bf16, tag="tp")
                    for t in range(4):
                        nc.tensor.transpose(
                            ot[:, t, 0 : E + 1],
                            t1s[hh][:, t * 128 : (t + 1) * 128],
                            ident[0 : E + 1, 0 : E + 1],
                        )
                    r = rpool.tile([128, 4], f32, tag="r")
                    nc.vector.reciprocal(r[:, :], ot[:, :, E])
                    nc.vector.tensor_mul(
                        o[:, :, hh, :], ot[:, :, 0:E], r.broadcast_to([128, 4, E])
                    )
                base = pair * L + qg * 512
                dst = od[base : base + 512, 2 * hp * E : (2 * hp + 2) * E].rearrange(
                    "(t p) (hh e) -> p t hh e", p=128, e=E
                )
                nc.sync.dma_start(dst, o[:, :, :, :])

            pending = [None]

            def compute_unit(pair, hp, qg):
                # Phase-separated: all QK+exp, then previous epilogue, then
                # all AV accumulation (v3.5 schedule, measured 151us/rep).
                njp = (4 * qg + 4) // 2
                ess = [emit_qk_exp(pair, hp, qg, jp) for jp in range(njp)]
                if pending[0] is not None:
                    unit_epilogue(*pending[0])
                a0 = psav.tile([E + 1, 512], f32, tag="av")
                a1 = psav.tile([E + 1, 512], f32, tag="av")
                avs = [a0, a1]
                for jp in range(njp):
                    emit_av(pair, hp, qg, avs, ess[jp], jp)
                pending[0] = (pair, hp, qg, avs)

            import contextlib

            loop_ctx = tc.For_i(0, reps) if reps > 1 else contextlib.nullcontext()
            with loop_ctx:
                alloc_slab(0)
                emit_slab_qk(0)
                nc.sync.dma_start(masks[:, :], maskd[:, :])
                emit_slab_v(0)

                for u in range(4):  # pair 0, qg0
                    compute_unit(0, u, 0)
                for u in range(4):  # pair 0, qg1 -- interleave pair-1 slab
                    if u == 0:
                        alloc_slab(1)
                        emit_slab_qk(1)
                    if u == 1:
                        emit_slab_v(1)
                    compute_unit(0, u, 1)
                for u in range(4):  # pair 1, qg0
                    compute_unit(1, u, 0)
                for u in range(4):  # pair 1, qg1
                    compute_unit(1, u, 1)
                unit_epilogue(*pending[0])
                pending[0] = None

    nc.compile()
    _CACHE[key] = nc
    if reps == 1:
        _CACHE["nc"] = nc
    return nc


def _shard(x):
    # [B, N, L, H, E] -> per-core [ROWS, HE] bf16 slices
    import ml_dtypes

    flat = np.asarray(x).astype(ml_dtypes.bfloat16).reshape(B * N, L, HE)
    return [
        np.ascontiguousarray(flat[c * PAIRS : (c + 1) * PAIRS].reshape(ROWS, HE))
        for c in range(CORES)
    ]


def kernel(queries, keys, values):
    from concourse.bass_utils import run_bass_kernel_spmd

    nc = _build()
    qs, ks, vs = _shard(queries), _shard(keys), _shard(values)
    in_maps = [
        {"queries": qs[c], "keys": ks[c], "values": vs[c]} for c in range(CORES)
    ]
    res = run_bass_kernel_spmd(nc, in_maps, core_ids=list(range(CORES)))
    out = np.concatenate(
        [res.results[c]["out"].reshape(PAIRS, L, H, E) for c in range(CORES)]
    )
    return np.ascontiguousarray(out.reshape(B, N, L, H, E))
